# revision 6
# baseline (speedup 1.0000x reference)
"""Self-contained Trainium2 Bass kernel for nn_Attention_20950850469901.

reference (per batch n):
    wv = v @ WV.T; wk = k @ WK.T; wq = q @ WQ.T
    scores = wq @ wk.T                                    [Sq, Sk]
    out = (softmax(scores, axis=q) / D) @ wv              [Sq, D]

Sharding: 8 cores = 4 batches x 2 key-halves. softmax is over the QUERY
axis, so splitting the KEY axis is embarrassingly parallel; the final
contraction over keys produces per-core partial sums that the host adds.

Math: scores = q @ (WQ.T @ WK) @ k.T = q @ A @ k.T with A precomputed on
host, evaluated as scoresT = (k_half @ A.T) @ q.T so the projection runs
over the SHARDED key side (half work per core, nothing duplicated):
    tT = A.T-tiles @ kT        (3-pass fp16 hi/lo, PSUM fp32)
    scoresT[k, q] = tT-tiles @ qT   (3-pass fp16 hi/lo)
    softmax along the free (q) axis, normalization folded into wv rows
    wv = v @ WV.T              (single-pass fp16)
    out = wqkT.T @ wv, scaled by 1/D on the PSUM->SBUF copy
fp16 hi/lo gives ~22 mantissa bits on the Q/K path so the (numerically
near-one-hot) softmax argmaxes match the fp32 reference (rel err ~3.5e-4).
"""

import os

os.environ.setdefault("JAX_PLATFORMS", "")

import numpy as np

N_B, S, D = 4, 2048, 1024
P = 128
NCORES = 8
SKH = S // 2  # keys per core
JT = D // P  # 8 contraction tiles (j axis, t-projection)
WT = D // P  # 8 w tiles
KHT = SKH // P  # 8 key tiles per core
KC = SKH // 512  # 2 key chunks of 512 (t-projection free dim)
QC = S // 512  # 4 query chunks of 512
QT = S // P  # 16 query tiles of 128
OC = D // 512  # 2 output chunks of 512

_CACHE = {}


def _build_nc(repeat=1):
    import concourse.bacc as bacc
    import concourse.mybir as mybir
    import concourse.tile as tile

    f16 = mybir.dt.float16
    f32 = mybir.dt.float32

    nc = bacc.Bacc(None, target_bir_lowering=False, debug=False)

    # DRAM inputs, host-prepped into [128, tiles, free] partition layouts.
    ATh = nc.dram_tensor("ATh", [P, JT, D], f16, kind="ExternalInput")
    ATl = nc.dram_tensor("ATl", [P, JT, D], f16, kind="ExternalInput")
    kTh = nc.dram_tensor("kTh", [P, JT, SKH], f16, kind="ExternalInput")
    kTl = nc.dram_tensor("kTl", [P, JT, SKH], f16, kind="ExternalInput")
    qTh = nc.dram_tensor("qTh", [P, WT, S], f16, kind="ExternalInput")
    qTl = nc.dram_tensor("qTl", [P, WT, S], f16, kind="ExternalInput")
    vTh = nc.dram_tensor("vTh", [P, WT, SKH], f16, kind="ExternalInput")
    WVTh = nc.dram_tensor("WVTh", [P, WT, D], f16, kind="ExternalInput")
    out = nc.dram_tensor("out", [S, D], f32, kind="ExternalOutput")

    with tile.TileContext(nc) as tc:
        with (
            tc.tile_pool(name="persist", bufs=1) as persist,
            tc.tile_pool(name="aw", bufs=2) as aw,       # ATh/ATl then vTh/WVTh
            tc.tile_pool(name="kstr", bufs=3) as kstr,   # kT chunks
            tc.tile_pool(name="small", bufs=4) as small,
            tc.tile_pool(name="outp", bufs=3) as outp,
            tc.tile_pool(name="ps_small", bufs=4, space="PSUM") as ps_small,
            tc.tile_pool(name="ps_sc", bufs=1, space="PSUM") as ps_sc,
        ):
            for _rep in range(repeat):
                # ---- resident tensors (144 KB/partition) ----
                q_h = persist.tile([P, WT, S], f16, tag="qTh")     # 32 KB
                q_l = persist.tile([P, WT, S], f16, tag="qTl")     # 32 KB
                tTh = persist.tile([P, WT, SKH], f16, tag="tTh")   # 16 KB
                tTl = persist.tile([P, WT, SKH], f16, tag="tTl")   # 16 KB
                wqkT = persist.tile([P, KHT, S], f16, tag="wqkT")  # 32 KB
                wv = persist.tile([P, KHT, D], f16, tag="wv")      # 16 KB
                recip = persist.tile([P, KHT], f32, tag="recip")

                # ---- stage T: tT[w, c] = sum_j A[w, j] * kT[j, c] ----
                a_h = aw.tile([P, JT, D], f16, tag="aw")
                a_l = aw.tile([P, JT, D], f16, tag="aw")
                k_c = {}
                # fine-grained first loads so PE starts ASAP
                for jt in range(JT):
                    nc.sync.dma_start(a_h[:, jt], ATh[:, jt])
                kc0h = kstr.tile([P, JT, 512], f16, tag="kc")
                for jt in range(JT):
                    nc.sync.dma_start(kc0h[:, jt], kTh[:, jt, 0:512])
                for jt in range(JT):
                    nc.sync.dma_start(a_l[:, jt], ATl[:, jt])
                kc0l = kstr.tile([P, JT, 512], f16, tag="kc")
                nc.sync.dma_start(kc0l[:], kTl[:, :, 0:512])
                k_c[0] = (kc0h, kc0l)

                for cc in range(KC):
                    if cc not in k_c:
                        kch = kstr.tile([P, JT, 512], f16, tag="kc")
                        kcl = kstr.tile([P, JT, 512], f16, tag="kc")
                        nc.sync.dma_start(kch[:], kTh[:, :, cc * 512 : (cc + 1) * 512])
                        nc.sync.dma_start(kcl[:], kTl[:, :, cc * 512 : (cc + 1) * 512])
                        k_c[cc] = (kch, kcl)
                    kch, kcl = k_c[cc]
                    for wt in range(WT):
                        ps = ps_small.tile([P, 512], f32, tag="ps_mm")
                        passes = ((a_h, kch), (a_h, kcl), (a_l, kch))
                        n_mm = len(passes) * JT
                        i = 0
                        for aa, kk in passes:
                            for jt in range(JT):
                                nc.tensor.matmul(
                                    ps[:],
                                    aa[:, jt, wt * P : (wt + 1) * P],
                                    kk[:, jt, :],
                                    start=(i == 0),
                                    stop=(i == n_mm - 1),
                                )
                                i += 1
                        # split psum into fp16 hi/lo
                        sl = np.s_[:, wt, cc * 512 : (cc + 1) * 512]
                        nc.scalar.copy(tTh[sl], ps[:])
                        nc.vector.tensor_tensor(
                            tTl[sl], ps[:], tTh[sl], mybir.AluOpType.subtract
                        )
                    if cc == 0:
                        # queue the big query loads behind the chunk-1 loads
                        kch1 = kstr.tile([P, JT, 512], f16, tag="kc")
                        kcl1 = kstr.tile([P, JT, 512], f16, tag="kc")
                        nc.sync.dma_start(kch1[:], kTh[:, :, 512:1024])
                        nc.sync.dma_start(kcl1[:], kTl[:, :, 512:1024])
                        k_c[1] = (kch1, kcl1)
                        nc.sync.dma_start(q_h[:], qTh[:])
                        nc.sync.dma_start(q_l[:], qTl[:])

                # ---- stage V loads (reuse AT slots; waits for stage T) ----
                v_h = aw.tile([P, WT, SKH], f16, tag="aw")
                wvt_h = aw.tile([P, WT, D], f16, tag="aw")
                nc.sync.dma_start(v_h[:], vTh[:])
                nc.sync.dma_start(wvt_h[:], WVTh[:])

                # ---- stage S (scores + softmax) with V projection interleaved ----
                for kt in range(KHT):
                    ps = ps_sc.tile([P, S], f32, tag="ps_sc")
                    for qc in range(QC):
                        passes = ((tTh, q_h), (tTl, q_h), (tTh, q_l))
                        n_mm = len(passes) * WT
                        i = 0
                        for tt, qq in passes:
                            for wt in range(WT):
                                nc.tensor.matmul(
                                    ps[:, qc * 512 : (qc + 1) * 512],
                                    tt[:, wt, kt * P : (kt + 1) * P],
                                    qq[:, wt, qc * 512 : (qc + 1) * 512],
                                    start=(i == 0),
                                    stop=(i == n_mm - 1),
                                )
                                i += 1
                    # softmax over q (free axis): per-partition (= per key)
                    negmax = small.tile([P, 1], f32, tag="negmax")
                    sums = small.tile([P, 1], f32, tag="sums")
                    nc.vector.tensor_reduce(
                        negmax[:], ps[:], axis=mybir.AxisListType.X,
                        op=mybir.AluOpType.max, negate=True,
                    )
                    nc.scalar.activation(
                        wqkT[:, kt, :], ps[:], mybir.ActivationFunctionType.Exp,
                        bias=negmax[:], accum_out=sums[:],
                    )
                    nc.vector.reciprocal(recip[:, kt : kt + 1], sums[:])

                    # V projection slice for this key tile: wv[kt] = vT.T @ WVT
                    for oc in range(OC):
                        psv = ps_small.tile([P, 512], f32, tag="ps_mm")
                        for wt in range(WT):
                            nc.tensor.matmul(
                                psv[:],
                                v_h[:, wt, kt * P : (kt + 1) * P],
                                wvt_h[:, wt, oc * 512 : (oc + 1) * 512],
                                start=(wt == 0),
                                stop=(wt == WT - 1),
                            )
                        nc.scalar.copy(wv[:, kt, oc * 512 : (oc + 1) * 512], psv[:])
                    # fold softmax normalization into wv rows (keys on partitions)
                    nc.vector.tensor_scalar_mul(
                        wv[:, kt, :], wv[:, kt, :], recip[:, kt : kt + 1]
                    )

                # ---- stage AV: out[q, o] = sum_k wqkT[k, q] * wv[k, o], /D ----
                for qt in range(QT):
                    for oc in range(OC):
                        ps = ps_small.tile([P, 512], f32, tag="ps_mm")
                        for kt in range(KHT):
                            nc.tensor.matmul(
                                ps[:],
                                wqkT[:, kt, qt * P : (qt + 1) * P],
                                wv[:, kt, oc * 512 : (oc + 1) * 512],
                                start=(kt == 0),
                                stop=(kt == KHT - 1),
                            )
                        ot = outp.tile([P, 512], f32, tag="ot")
                        nc.scalar.mul(ot[:], ps[:], 1.0 / D)
                        nc.sync.dma_start(
                            out[qt * P : (qt + 1) * P, oc * 512 : (oc + 1) * 512], ot[:]
                        )

    nc.compile()
    return nc


def _get_nc():
    if "nc" not in _CACHE:
        _CACHE["nc"] = _build_nc()
    return _CACHE["nc"]


def _split16(x):
    hi = x.astype(np.float16)
    lo = (x.astype(np.float32) - hi.astype(np.float32)).astype(np.float16)
    return hi, lo


def _part3(x2d):
    """[T*128, F] -> [128, T, F] with tile index t covering rows t*128+p."""
    t = x2d.shape[0] // P
    return np.ascontiguousarray(x2d.reshape(t, P, x2d.shape[1]).transpose(1, 0, 2))


def kernel(v, k, q, WV, WQ, WK):
    from concourse.bass_utils import run_bass_kernel_spmd

    v = np.asarray(v, dtype=np.float32)
    k = np.asarray(k, dtype=np.float32)
    q = np.asarray(q, dtype=np.float32)
    WV = np.asarray(WV, dtype=np.float32)
    WQ = np.asarray(WQ, dtype=np.float32)
    WK = np.asarray(WK, dtype=np.float32)

    A = (WQ.T.astype(np.float64) @ WK.astype(np.float64)).astype(np.float32)
    ATh, ATl = _split16(np.ascontiguousarray(A.T))
    ATh, ATl = _part3(ATh), _part3(ATl)
    WVTh = _part3(np.ascontiguousarray(WV.T).astype(np.float16))

    in_maps = []
    for c in range(NCORES):
        n, h = c // 2, c % 2
        qT = np.ascontiguousarray(q[n].T)
        qh, ql = _split16(qT)
        kT = np.ascontiguousarray(k[n, h * SKH : (h + 1) * SKH, :].T)
        kh, kl = _split16(kT)
        vT = np.ascontiguousarray(v[n, h * SKH : (h + 1) * SKH, :].T)
        in_maps.append(
            {
                "ATh": ATh,
                "ATl": ATl,
                "qTh": _part3(qh),
                "qTl": _part3(ql),
                "kTh": _part3(kh),
                "kTl": _part3(kl),
                "vTh": _part3(vT.astype(np.float16)),
                "WVTh": WVTh,
            }
        )

    nc = _get_nc()
    res = run_bass_kernel_spmd(nc, in_maps, core_ids=list(range(NCORES)))
    _CACHE["last_result"] = res
    out = np.zeros((N_B, S, D), dtype=np.float32)
    for n in range(N_B):
        out[n] = res.results[2 * n]["out"] + res.results[2 * n + 1]["out"]
    return out


# revision 9
# speedup vs baseline: 1.0858x; 1.0858x over previous
"""Self-contained Trainium2 Bass kernel for nn_Attention_20950850469901.

reference (per batch n):
    wv = v @ WV.T; wk = k @ WK.T; wq = q @ WQ.T
    scores = wq @ wk.T                                    [Sq, Sk]
    out = (softmax(scores, axis=q) / D) @ wv              [Sq, D]

Sharding: 8 cores = 4 batches x 2 key-halves. softmax is over the QUERY
axis, so splitting the KEY axis is embarrassingly parallel; the final
contraction over keys produces per-core partial sums that the host adds.

Math: scores = q @ (WQ.T @ WK) @ k.T = q @ A @ k.T with A precomputed on
host, evaluated as scoresT = (k_half @ A.T) @ q.T so the projection runs
over the SHARDED key side (half work per core, nothing duplicated):
    tT = A.T-tiles @ kT        (3-pass fp16 hi/lo, PSUM fp32)
    scoresT[k, q] = tT-tiles @ qT   (3-pass fp16 hi/lo)
    softmax along the free (q) axis, normalization folded into wv rows
    wv = v @ WV.T              (single-pass fp16)
    out = wqkT.T @ wv, scaled by 1/D on the PSUM->SBUF copy
fp16 hi/lo gives ~22 mantissa bits on the Q/K path so the (numerically
near-one-hot) softmax argmaxes match the fp32 reference (rel err ~3.5e-4).
"""

import os

os.environ.setdefault("JAX_PLATFORMS", "")

import numpy as np

N_B, S, D = 4, 2048, 1024
P = 128
NCORES = 8
SKH = S // 2  # keys per core
JT = D // P  # 8 contraction tiles (j axis, t-projection)
WT = D // P  # 8 w tiles
KHT = SKH // P  # 8 key tiles per core
KC = SKH // 512  # 2 key chunks of 512 (t-projection free dim)
QC = S // 512  # 4 query chunks of 512
QT = S // P  # 16 query tiles of 128
OC = D // 512  # 2 output chunks of 512

_CACHE = {}


def _build_nc(repeat=1, bare=False):
    import concourse.bacc as bacc
    import concourse.mybir as mybir
    import concourse.tile as tile

    f16 = mybir.dt.float16
    f32 = mybir.dt.float32

    nc = bacc.Bacc(None, target_bir_lowering=False, debug=False)

    # DRAM inputs, host-prepped into [128, tiles, free] partition layouts.
    ATh = nc.dram_tensor("ATh", [P, JT, D], f16, kind="ExternalInput")
    ATl = nc.dram_tensor("ATl", [P, JT, D], f16, kind="ExternalInput")
    kTh = nc.dram_tensor("kTh", [P, JT, SKH], f16, kind="ExternalInput")
    kTl = nc.dram_tensor("kTl", [P, JT, SKH], f16, kind="ExternalInput")
    qTh = nc.dram_tensor("qTh", [P, WT, S], f16, kind="ExternalInput")
    qTl = nc.dram_tensor("qTl", [P, WT, S], f16, kind="ExternalInput")
    vTh = nc.dram_tensor("vTh", [P, WT, SKH], f16, kind="ExternalInput")
    WVTh = nc.dram_tensor("WVTh", [P, WT, D], f16, kind="ExternalInput")
    out = nc.dram_tensor("out", [S, D], f32, kind="ExternalOutput")

    with tile.TileContext(nc) as tc:
        with (
            tc.tile_pool(name="persist", bufs=1) as persist,
            tc.tile_pool(name="aw", bufs=2) as aw,       # ATh/ATl then vTh/WVTh
            tc.tile_pool(name="kstr", bufs=3) as kstr,   # kT chunks
            tc.tile_pool(name="small", bufs=4) as small,
            tc.tile_pool(name="outp", bufs=3) as outp,
            tc.tile_pool(name="ps_small", bufs=4, space="PSUM") as ps_small,
            tc.tile_pool(name="ps_sc", bufs=1, space="PSUM") as ps_sc,
        ):
            for _rep in range(repeat):
                # ---- resident tensors (144 KB/partition) ----
                q_h = persist.tile([P, WT, S], f16, tag="qTh")     # 32 KB
                q_l = persist.tile([P, WT, S], f16, tag="qTl")     # 32 KB
                tTh = persist.tile([P, WT, SKH], f16, tag="tTh")   # 16 KB
                tTl = persist.tile([P, WT, SKH], f16, tag="tTl")   # 16 KB
                wqkT = persist.tile([P, KHT, S], f16, tag="wqkT")  # 32 KB
                wv = persist.tile([P, KHT, D], f16, tag="wv")      # 16 KB
                recip = persist.tile([P, KHT], f32, tag="recip")
                if bare:
                    for t_ in (tTh, tTl, wqkT, wv):
                        nc.vector.memset(t_[:], 0.25)
                    nc.vector.memset(recip[:], 1.0)

                # ---- stage T: tT[w, c] = sum_j A[w, j] * kT[j, c] ----
                a_h = aw.tile([P, JT, D], f16, tag="aw")
                a_l = aw.tile([P, JT, D], f16, tag="aw")
                k_c = {}
                # fine-grained first loads so PE starts ASAP
                for jt in range(JT):
                    nc.sync.dma_start(a_h[:, jt], ATh[:, jt])
                kc0h = kstr.tile([P, JT, 512], f16, tag="kc")
                for jt in range(JT):
                    nc.sync.dma_start(kc0h[:, jt], kTh[:, jt, 0:512])
                for jt in range(JT):
                    nc.sync.dma_start(a_l[:, jt], ATl[:, jt])
                kc0l = kstr.tile([P, JT, 512], f16, tag="kc")
                nc.sync.dma_start(kc0l[:], kTl[:, :, 0:512])
                k_c[0] = (kc0h, kc0l)

                for cc in range(KC):
                    if cc not in k_c:
                        kch = kstr.tile([P, JT, 512], f16, tag="kc")
                        kcl = kstr.tile([P, JT, 512], f16, tag="kc")
                        nc.sync.dma_start(kch[:], kTh[:, :, cc * 512 : (cc + 1) * 512])
                        nc.sync.dma_start(kcl[:], kTl[:, :, cc * 512 : (cc + 1) * 512])
                        k_c[cc] = (kch, kcl)
                    kch, kcl = k_c[cc]
                    for wt in range(WT):
                        ps = ps_small.tile([P, 512], f32, tag="ps_mm")
                        passes = ((a_h, kch), (a_h, kcl), (a_l, kch))
                        n_mm = len(passes) * JT
                        i = 0
                        for aa, kk in passes:
                            for jt in range(JT):
                                nc.tensor.matmul(
                                    ps[:],
                                    aa[:, jt, wt * P : (wt + 1) * P],
                                    kk[:, jt, :],
                                    start=(i == 0),
                                    stop=(i == n_mm - 1),
                                )
                                i += 1
                        # split psum into fp16 hi/lo
                        if bare:
                            nc.scalar.copy(tTh[:, wt, cc * 512 : cc * 512 + 1], ps[:, 0:1])
                        else:
                            sl = np.s_[:, wt, cc * 512 : (cc + 1) * 512]
                            nc.scalar.copy(tTh[sl], ps[:])
                            nc.vector.tensor_tensor(
                                tTl[sl], ps[:], tTh[sl], mybir.AluOpType.subtract
                            )
                    if cc == 0:
                        # queue the big query loads behind the chunk-1 loads
                        kch1 = kstr.tile([P, JT, 512], f16, tag="kc")
                        kcl1 = kstr.tile([P, JT, 512], f16, tag="kc")
                        nc.sync.dma_start(kch1[:], kTh[:, :, 512:1024])
                        nc.sync.dma_start(kcl1[:], kTl[:, :, 512:1024])
                        k_c[1] = (kch1, kcl1)
                        nc.sync.dma_start(q_h[:], qTh[:])
                        nc.sync.dma_start(q_l[:], qTl[:])

                # ---- stage V loads (reuse AT slots; waits for stage T) ----
                v_h = aw.tile([P, WT, SKH], f16, tag="aw")
                wvt_h = aw.tile([P, WT, D], f16, tag="aw")
                nc.sync.dma_start(v_h[:], vTh[:])
                nc.sync.dma_start(wvt_h[:], WVTh[:])

                # ---- stage S (scores + softmax) with V projection interleaved ----
                for kt in range(KHT):
                    ps = ps_sc.tile([P, S], f32, tag="ps_sc")
                    for qc in range(QC):
                        passes = ((tTh, q_h), (tTl, q_h), (tTh, q_l))
                        n_mm = len(passes) * WT
                        i = 0
                        for tt, qq in passes:
                            for wt in range(WT):
                                nc.tensor.matmul(
                                    ps[:, qc * 512 : (qc + 1) * 512],
                                    tt[:, wt, kt * P : (kt + 1) * P],
                                    qq[:, wt, qc * 512 : (qc + 1) * 512],
                                    start=(i == 0),
                                    stop=(i == n_mm - 1),
                                )
                                i += 1
                    # softmax over q (free axis): per-partition (= per key)
                    if bare:
                        nc.scalar.copy(wqkT[:, kt, 0:1], ps[:, 0:1])
                    else:
                        negmax = small.tile([P, 1], f32, tag="negmax")
                        sums = small.tile([P, 1], f32, tag="sums")
                        nc.vector.tensor_reduce(
                            negmax[:], ps[:], axis=mybir.AxisListType.X,
                            op=mybir.AluOpType.max, negate=True,
                        )
                        nc.scalar.activation(
                            wqkT[:, kt, :], ps[:], mybir.ActivationFunctionType.Exp,
                            bias=negmax[:], accum_out=sums[:],
                        )
                        nc.vector.reciprocal(recip[:, kt : kt + 1], sums[:])

                    # V projection slice for this key tile: wv[kt] = vT.T @ WVT
                    for oc in range(OC):
                        psv = ps_small.tile([P, 512], f32, tag="ps_mm")
                        for wt in range(WT):
                            nc.tensor.matmul(
                                psv[:],
                                v_h[:, wt, kt * P : (kt + 1) * P],
                                wvt_h[:, wt, oc * 512 : (oc + 1) * 512],
                                start=(wt == 0),
                                stop=(wt == WT - 1),
                            )
                        if bare:
                            nc.scalar.copy(wv[:, kt, oc * 512 : oc * 512 + 1], psv[:, 0:1])
                        else:
                            nc.scalar.copy(wv[:, kt, oc * 512 : (oc + 1) * 512], psv[:])
                    if not bare:
                        # fold softmax normalization into wv rows (keys on partitions)
                        nc.vector.tensor_scalar_mul(
                            wv[:, kt, :], wv[:, kt, :], recip[:, kt : kt + 1]
                        )

                # ---- stage AV: out[q, o] = sum_k wqkT[k, q] * wv[k, o], /D ----
                for qt in range(QT):
                    for oc in range(OC):
                        ps = ps_small.tile([P, 512], f32, tag="ps_mm")
                        for kt in range(KHT):
                            nc.tensor.matmul(
                                ps[:],
                                wqkT[:, kt, qt * P : (qt + 1) * P],
                                wv[:, kt, oc * 512 : (oc + 1) * 512],
                                start=(kt == 0),
                                stop=(kt == KHT - 1),
                            )
                        ot = outp.tile([P, 512], f32, tag="ot")
                        nc.scalar.mul(ot[:], ps[:], 1.0 / D)
                        nc.sync.dma_start(
                            out[qt * P : (qt + 1) * P, oc * 512 : (oc + 1) * 512], ot[:]
                        )

    nc.compile()
    return nc


def _get_nc():
    if "nc" not in _CACHE:
        _CACHE["nc"] = _build_nc()
    return _CACHE["nc"]


def _split16(x):
    hi = x.astype(np.float16)
    lo = (x.astype(np.float32) - hi.astype(np.float32)).astype(np.float16)
    return hi, lo


def _part3(x2d):
    """[T*128, F] -> [128, T, F] with tile index t covering rows t*128+p."""
    t = x2d.shape[0] // P
    return np.ascontiguousarray(x2d.reshape(t, P, x2d.shape[1]).transpose(1, 0, 2))


def kernel(v, k, q, WV, WQ, WK):
    from concourse.bass_utils import run_bass_kernel_spmd

    v = np.asarray(v, dtype=np.float32)
    k = np.asarray(k, dtype=np.float32)
    q = np.asarray(q, dtype=np.float32)
    WV = np.asarray(WV, dtype=np.float32)
    WQ = np.asarray(WQ, dtype=np.float32)
    WK = np.asarray(WK, dtype=np.float32)

    A = (WQ.T.astype(np.float64) @ WK.astype(np.float64)).astype(np.float32)
    ATh, ATl = _split16(np.ascontiguousarray(A.T))
    ATh, ATl = _part3(ATh), _part3(ATl)
    WVTh = _part3(np.ascontiguousarray(WV.T).astype(np.float16))

    in_maps = []
    for c in range(NCORES):
        n, h = c // 2, c % 2
        qT = np.ascontiguousarray(q[n].T)
        qh, ql = _split16(qT)
        kT = np.ascontiguousarray(k[n, h * SKH : (h + 1) * SKH, :].T)
        kh, kl = _split16(kT)
        vT = np.ascontiguousarray(v[n, h * SKH : (h + 1) * SKH, :].T)
        in_maps.append(
            {
                "ATh": ATh,
                "ATl": ATl,
                "qTh": _part3(qh),
                "qTl": _part3(ql),
                "kTh": _part3(kh),
                "kTl": _part3(kl),
                "vTh": _part3(vT.astype(np.float16)),
                "WVTh": WVTh,
            }
        )

    nc = _get_nc()
    res = run_bass_kernel_spmd(nc, in_maps, core_ids=list(range(NCORES)))
    _CACHE["last_result"] = res
    out = np.zeros((N_B, S, D), dtype=np.float32)
    for n in range(N_B):
        out[n] = res.results[2 * n]["out"] + res.results[2 * n + 1]["out"]
    return out
